# revision 1
# baseline (speedup 1.0000x reference)
"""Additive attention, query-position-sharded Bass kernel for 8 TRN2 cores.

Sharding: each core owns 1/8 of every batch's valid q-range
(qc_b = ceil(vl_b/8) columns, identical across cores -> near-perfect SPMD
balance at ~sum(vl)/8 columns x 256 k per core). Softmax over q is split
across cores: each core emits unnormalized partial attn@value and a
partial denominator; the host sums partials and divides (exact).

Per-core pipeline (per q column j of batch b):
  DVE:  sum[:, slot, :] = kT_b(bf16) + qT[:, j]   (tensor_scalar add, 256 free)
  ACT:  tanh in-place over tapered chunks (one big ACTIVATE per chunk)
  PE:   per column x 2 k-blocks: score col = feat^T @ wv into PSUM
  ACT:  exp over grouped batches' score tiles
  PE:   attn transpose, attn^T @ value (AV), attn^T @ colmask (den)
Masked (padded) columns are neutralized by host-zeroed value rows (AV)
and the 0/1 column mask (den) -- no on-device masking.
"""

import numpy as np
import ml_dtypes

import concourse.bass as bass
import concourse.bacc as bacc
import concourse.tile as tile
from concourse import mybir
from concourse.bass_utils import run_bass_kernel_spmd

B = 16
NK = 256
NQ = 256
DK = 256
DV = 256
H = 128
P = 128
NCORES = 8
CH = 64          # max q-columns per tanh chunk
QCMAX = 32       # max cols per (batch, core)
GROUP_F32 = 128  # f32 columns per score-psum group tile

F32 = mybir.dt.float32
BF16 = mybir.dt.bfloat16
TANH = mybir.ActivationFunctionType.Tanh
EXP = mybir.ActivationFunctionType.Exp

BF = ml_dtypes.bfloat16
USE_XBAR = False

_CACHE = {}


def _plan(qc):
    desc = sorted(range(B), key=lambda b: (-qc[b], b))
    # interleave big/small so score groups complete evenly through the
    # stream; ends on the smallest batch for a short drain
    order = []
    for i in range(B // 2):
        order += [desc[i], desc[i + B // 2]]
    cols = []  # (batch, j, qT column index)
    off = 0
    for b in order:
        for j in range(qc[b]):
            cols.append((b, j, off + j))
        off += qc[b]
    ncols = off
    # score groups: consecutive batches, sum(2*qc) <= GROUP_F32; last four
    # batches ride solo so the drain pipeline is fine-grained
    groups = []
    cur, cur_sz = [], 0
    for b in order[:-2]:
        if cur and cur_sz + 2 * qc[b] > GROUP_F32:
            groups.append(cur)
            cur, cur_sz = [], 0
        cur.append(b)
        cur_sz += 2 * qc[b]
    if cur:
        groups.append(cur)
    for b in order[-2:]:
        groups.append([b])
    gidx, gbase = {}, {}
    for gi, g in enumerate(groups):
        boff = 0
        for b in g:
            gidx[b] = gi
            gbase[b] = boff
            boff += 2 * qc[b]
    return order, cols, ncols, groups, gidx, gbase


def _chunk_sizes(n):
    sizes = []
    for r in (8, 12, 16, 20, 28):
        if sum(sizes) + r <= n:
            sizes.append(r)
    while n - sum(sizes) > 48:
        sizes.append(32)
    rem = n - sum(sizes)
    if rem > 8:
        sizes += [rem - 8, 8]
    elif rem > 0:
        sizes.append(rem)
    return sizes


def _build(qc):
    order, cols, ncols, groups, gidx, gbase = _plan(qc)

    nc = bacc.Bacc("TRN2", target_bir_lowering=False, debug=False,
                   num_devices=NCORES)

    keyT_d = nc.dram_tensor("keyT", [B, P, 2, NK], BF16, kind="ExternalInput")
    qryT_d = nc.dram_tensor("qryT", [P, 2, ncols], BF16, kind="ExternalInput")
    val_d = nc.dram_tensor("val", [QCMAX, B, DV], BF16, kind="ExternalInput")
    mask_d = nc.dram_tensor("maskP", [QCMAX, B], BF16, kind="ExternalInput")
    wk_d = nc.dram_tensor("Wk", [P, 2, H], BF16, kind="ExternalInput")
    wq_d = nc.dram_tensor("Wq", [P, 2, H], BF16, kind="ExternalInput")
    wv_d = nc.dram_tensor("wv", [H, 1], BF16, kind="ExternalInput")
    id_d = nc.dram_tensor("ident", [P, P], BF16, kind="ExternalInput")
    av_d = nc.dram_tensor("av", [B, P, 2, DV], BF16, kind="ExternalOutput")
    den_d = nc.dram_tensor("den", [P, 2 * B], F32, kind="ExternalOutput")

    with tile.TileContext(nc) as tc:
        with (
            tc.tile_pool(name="const", bufs=1) as const,
            tc.tile_pool(name="big", bufs=1) as big,
            tc.tile_pool(name="kin", bufs=2) as kin,
            tc.tile_pool(name="sumr", bufs=3) as sumr,
            tc.tile_pool(name="attnp", bufs=2) as attnp,
            tc.tile_pool(name="atp", bufs=4) as atp,
            tc.tile_pool(name="ps_proj", bufs=1, space="PSUM") as ps_proj,
            tc.tile_pool(name="ps_sc", bufs=2, space="PSUM") as ps_sc,
            tc.tile_pool(name="ps_av", bufs=2, space="PSUM") as ps_av,
            tc.tile_pool(name="ps_den", bufs=1, space="PSUM") as ps_den,
        ):
            wk_sb = const.tile([P, 2, H], BF16)
            wq_sb = const.tile([P, 2, H], BF16)
            wv_sb = const.tile([H, 1], BF16)
            id_sb = const.tile([P, P], BF16)
            qryT_sb = const.tile([P, 2, ncols], BF16)
            val_sb = const.tile([QCMAX, B, DV], BF16)
            mask_sb = const.tile([QCMAX, B], BF16)

            kT_sb = big.tile([P, B, NK], BF16)   # h-major projected keys
            qT_sb = big.tile([P, ncols], F32)    # h-major projected queries

            # ---- input DMAs (first-need first) ----
            keyT_tiles = {}
            _kq = [0]

            def load_key(b):
                t = kin.tile([P, 2, NK], BF16, name=f"keyT{b}", tag="keyT")
                keyT_tiles[b] = t
                eng = nc.sync if (_kq[0] % 2 == 0) else nc.gpsimd
                _kq[0] += 1
                eng.dma_start(out=t, in_=keyT_d[b, :, :, :])

            nc.sync.dma_start(out=qryT_sb[:, :, :32], in_=qryT_d[:, :, :32])
            nc.gpsimd.dma_start(out=wk_sb, in_=wk_d[:, :, :])
            load_key(order[0])        # sync
            nc.gpsimd.dma_start(out=wq_sb, in_=wq_d[:, :, :])
            load_key(order[1])        # gpsimd
            nc.gpsimd.dma_start(out=wv_sb, in_=wv_d[:, :])
            nc.gpsimd.dma_start(out=id_sb, in_=id_d[:, :])
            nc.gpsimd.dma_start(out=mask_sb, in_=mask_d[:, :])
            nc.gpsimd.dma_start(out=val_sb, in_=val_d[:, :, :])

            projected = set()

            def kproj(b):
                projected.add(b)
                kp = ps_proj.tile([P, NK], F32, name=f"kproj{b}", tag="proj")
                for dk in range(2):
                    nc.tensor.matmul(kp, wk_sb[:, dk, :],
                                     keyT_tiles[b][:, dk, :],
                                     start=(dk == 0), stop=(dk == 1))
                if 1 <= order.index(b) < 9:
                    nc.scalar.copy(kT_sb[:, b, :], kp)
                else:
                    nc.vector.tensor_copy(kT_sb[:, b, :], kp)

            kproj(order[0])

            nc.sync.dma_start(out=qryT_sb[:, :, 32:], in_=qryT_d[:, :, 32:])

            # ---- q projection: qT[h, c] = sum_dk Wq[dk,h] * qryT[dk,c] ----
            qp = ps_proj.tile([P, ncols], F32, name="qproj", tag="proj")
            for dk in range(2):
                nc.tensor.matmul(qp[:, :32], wq_sb[:, dk, :],
                                 qryT_sb[:, dk, :32],
                                 start=(dk == 0), stop=(dk == 1))
            nc.vector.tensor_copy(qT_sb[:, :32], qp[:, :32])
            for dk in range(2):
                nc.tensor.matmul(qp[:, 32:], wq_sb[:, dk, :],
                                 qryT_sb[:, dk, 32:],
                                 start=(dk == 0), stop=(dk == 1))
            nc.vector.tensor_copy(qT_sb[:, 32:], qp[:, 32:])

            kproj(order[1])

            # ---- score psum groups ----
            gtiles = [
                ps_sc.tile([P, GROUP_F32], F32, name=f"sg{gi}", tag="sg")
                for gi in range(len(groups))
            ]
            den_ps = ps_den.tile([P, 2 * B], F32)
            last_col = {}
            for ci_, (b, j, qi) in enumerate(cols):
                last_col[b] = ci_
            group_done_at = {}
            for gi, g in enumerate(groups):
                group_done_at[max(last_col[b] for b in g)] = gi

            deferred = []  # (due_chunk, closure), kept due-sorted
            cur_ci = [0]
            _avq = [0]

            def push(due, fn):
                deferred.append((due, fn))
                deferred.sort(key=lambda t: t[0])

            def drain(force=False):
                while deferred and (force or deferred[0][0] <= cur_ci[0]):
                    deferred.pop(0)[1]()
                    if not force:
                        break

            def emit_epilogue(gi, ci):
                g = groups[gi]
                gt = gtiles[gi]
                used = sum(2 * qc[b] for b in g)

                def stage_a():
                    attn_sb = attnp.tile([P, GROUP_F32], BF16,
                                         name=f"attn{gi}", tag="attn")
                    nc.scalar.activation(out=attn_sb[:, :used],
                                         in_=gt[:, :used], func=EXP)
                    for b in g:
                        n = qc[b]
                        tp = ps_av.tile([P, 2, P], BF16, name=f"tp{b}",
                                        tag="tp")
                        for kb in range(2):
                            sl = attn_sb[:, gbase[b] + kb * n:
                                         gbase[b] + (kb + 1) * n]
                            nc.tensor.transpose(tp[:n, kb, :], sl, id_sb)
                        push(ci + 3, stage_b1(b, tp))

                def stage_b1(b, tp):
                    def go():
                        n = qc[b]
                        at = atp.tile([P, 2, P], BF16, name=f"at{b}", tag="at")
                        nc.vector.tensor_copy(at[:n, :, :], tp[:n, :, :])
                        av = ps_av.tile([P, 2, DV], F32, name=f"av{b}",
                                        tag="av")
                        for kb in range(2):
                            nc.tensor.matmul(av[:, kb, :], at[:n, kb, :],
                                             val_sb[:, b, :][0:n, :],
                                             start=True, stop=True)
                            nc.tensor.matmul(
                                den_ps[:, 2 * b + kb: 2 * b + kb + 1],
                                at[:n, kb, :], mask_sb[:, b: b + 1][0:n, :],
                                start=True, stop=True)
                        def go2():
                            av_sb = atp.tile([P, 2, DV], BF16,
                                             name=f"avs{b}", tag="avs")
                            if gidx[b] >= len(groups) - 3:
                                nc.scalar.copy(av_sb, av)
                            else:
                                nc.vector.tensor_copy(av_sb, av)
                            eng = nc.sync if (_avq[0] % 2 == 0) else nc.gpsimd
                            _avq[0] += 1
                            eng.dma_start(out=av_d[b, :, :, :], in_=av_sb)
                        push(ci + 4, go2)
                    return go

                push(ci + 1, stage_a)

            # ---- main column stream ----
            sizes = _chunk_sizes(len(cols))
            starts = [sum(sizes[:i]) for i in range(len(sizes))]
            for ci, (c0, csz) in enumerate(zip(starts, sizes)):
                cur_ci[0] = ci
                chunk = cols[c0: c0 + csz]
                nxt = sizes[ci + 1] if ci + 1 < len(sizes) else 0
                horizon = {b for (b, j, qi) in
                           cols[c0: c0 + csz + nxt] if j == 0}
                for b in order:
                    if b in horizon and b not in keyT_tiles:
                        load_key(b)
                for b in order:
                    if b in horizon and b not in projected:
                        kproj(b)

                st = sumr.tile([P, CH, NK], BF16, name=f"sum{ci}", tag="sum")
                for si, (b, j, qi) in enumerate(chunk):
                    nc.vector.tensor_scalar_add(
                        out=st[:, si, :], in0=kT_sb[:, b, :],
                        scalar1=qT_sb[:, qi: qi + 1])
                    if si % 2 == 1 and 3 * si >= 2 * csz:
                        drain()
                g = len(chunk)
                nc.scalar.activation(out=st[:, :g, :], in_=st[:, :g, :],
                                     func=TANH)
                for si, (b, j, qi) in enumerate(chunk):
                    gt = gtiles[gidx[b]]
                    base = gbase[b]
                    n = qc[b]
                    for kb in range(2):
                        nc.tensor.matmul(
                            gt[:, base + kb * n + j: base + kb * n + j + 1],
                            st[:, si, kb * P: (kb + 1) * P], wv_sb,
                            start=True, stop=True)
                for done_at, gi in sorted(group_done_at.items()):
                    if c0 <= done_at <= c0 + csz - 1:
                        emit_epilogue(gi, ci)

            drain(force=True)
            den_sb = const.tile([P, 2 * B], F32)
            nc.scalar.copy(den_sb, den_ps)
            nc.sync.dma_start(out=den_d[:, :], in_=den_sb)

    nc.compile()
    return nc


def kernel(key, query, value, valid_lens, Wk, Wq, wv, _trace=False):
    key = np.asarray(key, dtype=np.float32)
    query = np.asarray(query, dtype=np.float32)
    value = np.asarray(value, dtype=np.float32)
    valid_lens = np.asarray(valid_lens)
    Wk = np.asarray(Wk, dtype=np.float32)
    Wq = np.asarray(Wq, dtype=np.float32)
    wv = np.asarray(wv, dtype=np.float32)

    vl = np.clip(valid_lens.astype(np.int64), 1, NQ)
    qc = [int(-(-v // NCORES)) for v in vl]
    qkey = tuple(qc)
    if qkey not in _CACHE:
        _CACHE[qkey] = _build(qc)
    nc = _CACHE[qkey]
    order, cols, ncols, groups, gidx, gbase = _plan(qc)

    keyT = np.ascontiguousarray(
        key.transpose(0, 2, 1).reshape(B, 2, P, NK).transpose(0, 2, 1, 3)
    ).astype(BF)
    wk_h = np.ascontiguousarray(Wk.reshape(2, P, H).transpose(1, 0, 2)).astype(BF)
    wq_h = np.ascontiguousarray(Wq.reshape(2, P, H).transpose(1, 0, 2)).astype(BF)
    wv_h = wv.reshape(H, 1).astype(BF)
    id_h = np.eye(P, dtype=np.float32).astype(BF)

    in_maps = []
    for c in range(NCORES):
        qryT = np.zeros((DK, ncols), dtype=np.float32)
        valp = np.zeros((QCMAX, B, DV), dtype=np.float32)
        maskp = np.zeros((QCMAX, B), dtype=np.float32)
        off = 0
        for b in order:
            n = qc[b]
            lo = c * n
            rows = query[b, lo: lo + n, :]          # (n, DK)
            qryT[:, off: off + n] = rows.T
            nvalid = int(np.clip(vl[b] - lo, 0, n))
            if nvalid > 0:
                valp[:nvalid, b, :] = value[b, lo: lo + nvalid, :]
                maskp[:nvalid, b] = 1.0
            off += n
        in_maps.append({
            "keyT": keyT,
            "qryT": np.ascontiguousarray(
                qryT.reshape(2, P, ncols).transpose(1, 0, 2)).astype(BF),
            "val": valp.astype(BF),
            "maskP": maskp.astype(BF),
            "Wk": wk_h,
            "Wq": wq_h,
            "wv": wv_h,
            "ident": id_h,
        })

    res = run_bass_kernel_spmd(nc, in_maps, core_ids=list(range(NCORES)),
                               trace=_trace)
    kernel.last_results = res

    av = np.zeros((B, P, 2, DV), dtype=np.float64)
    den = np.zeros((P, 2 * B), dtype=np.float64)
    for c in range(NCORES):
        av += np.asarray(res.results[c]["av"], dtype=np.float64)
        den += np.asarray(res.results[c]["den"], dtype=np.float64)
    out = np.empty((B, NK, DV), dtype=np.float32)
    for b in range(B):
        for kb in range(2):
            d = den[:, 2 * b + kb]              # (128,)
            out[b, kb * P: (kb + 1) * P, :] = (
                av[b, :, kb, :] / d[:, None]).astype(np.float32)
    return out



# revision 8
# speedup vs baseline: 2.4583x; 2.4583x over previous
"""Additive attention via rank-R separable tanh expansion, batch-sharded
over 8 TRN2 cores (2 batches per core).

Key identity: tanh(a+b) is a smooth symmetric bivariate function, so
  tanh(k_h + q_h) ~= sum_r (c_r*tanh(s_r*k_h + t_r) + be_r) * tanh(p_r*q_h + w_r)
(rank R=14 fit, Gaussian-weighted; weighted-RMS ~6.6e-3). The huge
(NK,NQ,H) tanh cube of the direct algorithm collapses into:
  scores[k,q] = sum_{h,r} KFW_r[h,k] * QF_r[h,q]     (a TensorE matmul)
where KFW_r = (c_r*tanh(s_r*kx + t_r) + be_r)*wv_h (k-features, scaled)
and   QF_r  = tanh(p_r*qx + w_r)                    (q-features, raw),
so ScalarE evaluates tanh only on the SMALL projected tensors
(R*(512+N0+N1) elems/core instead of ~8.4M).

Per-core pipeline:
  PE:   kx/qx = Wk/Wq projections (PSUM)
  DVE:  copy projections into X; per r: affine s_r*x+t_r into FT;
        per r: KFW = FT_k*(c_r*wv) + be_r*wv  (per-partition AP scalars)
  ACT:  tanh in-place over FT in r-groups (one big ACTIVATE per group)
  PE:   scoresT[q,k] accumulated over r per (batch, q-block)
  ACT:  exp (PSUM->SBUF, bf16)
  PE:   attnT^T @ [value | ones] -> av + den (f32)
Softmax denominator rides as value column 256; host divides in f64.
Masking: q-feature columns beyond valid_len get zero value rows and a
zero ones-column entry (host-prepared), so they contribute nothing.
SPMD: all cores run one program shaped (N0, N1) = padded max pair
valid-lens; batches paired big+small for load balance.
"""

import numpy as np
import ml_dtypes

import concourse.bass as bass
import concourse.bacc as bacc
import concourse.tile as tile
from concourse import mybir
from concourse.bass_utils import run_bass_kernel_spmd

B = 16
NK = 256
NQ = 256
DK = 256
DV = 256
H = 128
P = 128
NCORES = 8

F32 = mybir.dt.float32
BF16 = mybir.dt.bfloat16
TANH = mybir.ActivationFunctionType.Tanh
EXP = mybir.ActivationFunctionType.Exp
MULT = mybir.AluOpType.mult
ADD = mybir.AluOpType.add

BF = ml_dtypes.bfloat16

# rank-R separable fit of tanh(a+b), rows = (c, s, t, be, p, w):
# tanh(a+b) ~= sum_r (c_r*tanh(s_r*a + t_r) + be_r) * tanh(p_r*b + w_r)
PARAMS = (
    (-2.52040568e-01, 1.99002666e+00, -4.47899878e+00, -9.42549801e-02, 2.15197328e+00, 3.09373387e+00),
    (3.64636150e-01, 1.58516661e+00, -2.07761486e+00, 1.42204763e-01, 1.44414533e+00, 2.89714946e+00),
    (-2.47950127e-01, 1.94387483e+00, -2.78516160e+00, -1.74826733e-02, 2.05517456e+00, 1.62082483e+00),
    (-3.28380385e-03, 1.16660353e+01, -4.36530701e+00, 1.70579797e-01, 1.72804735e+00, 3.74842149e+00),
    (-2.56904980e-01, 1.77145780e+00, 3.86932251e+00, 8.74206769e-04, 9.03416015e-01, -4.09168863e+00),
    (3.14097109e-01, 1.64022510e+00, 1.78609555e+00, -2.09084566e-02, 1.75557271e+00, -8.61302366e-01),
    (3.63810889e-01, 1.46289509e+00, 2.98283535e+00, -6.67224595e-03, 1.77132556e+00, -2.30473105e+00),
    (3.20196965e-01, 1.73064180e+00, -7.82208088e-01, 4.81429713e-02, 1.58086949e+00, 1.66439422e+00),
    (3.14926851e-01, 1.68186675e+00, 5.03609667e-01, -9.32675588e-04, 1.71954321e+00, 4.53667119e-01),
    (-3.00941962e-01, 1.73112294e+00, -1.19426368e+00, 1.03400799e-02, 1.89247073e+00, 2.12824057e-01),
    (-2.94329673e-01, 1.73093804e+00, 3.49751572e-02, 1.87094887e-02, 1.85537290e+00, -1.11772337e+00),
    (-2.74705215e-01, 1.74670793e+00, 1.24557836e+00, -3.42475503e-02, 1.94228525e+00, -2.59242166e+00),
    (-2.80191891e-01, 1.67858968e+00, 2.34991773e+00, 2.56972624e-01, 1.90945375e+00, -4.11460695e+00),
    (-2.81152597e-01, 1.84197190e+00, -4.11615444e+00, 1.48669857e-02, 4.37983764e-01, -2.02003138e+00),
)
R = len(PARAMS)
RGROUPS = (4, 4, 3, 3)   # ScalarE tanh chunking over units

_CACHE = {}


def _slots(N0, N1):
    """q-block slots: list of (batch_idx 0/1, qb, nn, qcol_offset_in_X)."""
    out = []
    for bi, (N, base) in enumerate(((N0, 512), (N1, 512 + N0))):
        nqb = (N + P - 1) // P
        for qb in range(nqb):
            nn = min(P, N - qb * P)
            out.append((bi, qb, nn, base + qb * P))
    return out


def _build(N0, N1, debug=False):
    NQT = N0 + N1
    L = 512 + NQT
    slots = _slots(N0, N1)
    nqb = len(slots)
    nq = [sum(1 for s in slots if s[0] == bi) for bi in (0, 1)]

    nc = bacc.Bacc("TRN2", target_bir_lowering=False, debug=False,
                   num_devices=NCORES)

    keyT_d = nc.dram_tensor("keyT", [2, P, 2, NK], BF16, kind="ExternalInput")
    qryT_d = nc.dram_tensor("qryT", [P, 2, NQT], BF16, kind="ExternalInput")
    val_d = nc.dram_tensor("val", [P, nqb, DV + 1], BF16, kind="ExternalInput")
    wk_d = nc.dram_tensor("Wk", [P, 2, H], BF16, kind="ExternalInput")
    wq_d = nc.dram_tensor("Wq", [P, 2, H], BF16, kind="ExternalInput")
    wvc_d = nc.dram_tensor("wvc", [P, 2, R], F32, kind="ExternalInput")
    av_d = nc.dram_tensor("av", [2, 2, P, DV + 1], F32, kind="ExternalOutput")
    if debug:
        xdbg_d = nc.dram_tensor("Xdbg", [P, L], BF16, kind="ExternalOutput")
        ftdbg_d = nc.dram_tensor("FTdbg", [P, R, L], BF16,
                                 kind="ExternalOutput")
        kfwdbg_d = nc.dram_tensor("KFWdbg", [P, R, 2 * NK], BF16,
                                  kind="ExternalOutput")
        scdbg_d = nc.dram_tensor("SCdbg", [P, nqb, NK], F32,
                                 kind="ExternalOutput")
        atdbg_d = nc.dram_tensor("ATdbg", [P, nqb, NK], BF16,
                                 kind="ExternalOutput")

    with tile.TileContext(nc) as tc:
        with (
            tc.tile_pool(name="const", bufs=1) as const,
            tc.tile_pool(name="ps_proj", bufs=1, space="PSUM") as ps_proj,
            tc.tile_pool(name="ps_sc", bufs=1, space="PSUM") as ps_sc,
            tc.tile_pool(name="ps_av", bufs=1, space="PSUM") as ps_av,
        ):
            wk_sb = const.tile([P, 2, H], BF16)
            wq_sb = const.tile([P, 2, H], BF16)
            wvc_sb = const.tile([P, 2, R], F32)
            kin = const.tile([P, 2, 2, NK], BF16)     # (b, dkblk, k)
            qin = const.tile([P, 2, NQT], BF16)       # (dkblk, q)
            val_sb = const.tile([P, nqb, DV + 1], BF16)
            X = const.tile([P, L], BF16)              # [kx0|kx1|qx0|qx1]
            FT = const.tile([P, R, L], BF16)          # affine then tanh in place
            KFW = const.tile([P, R, 2 * NK], BF16)    # scaled k-features
            attnT = const.tile([P, nqb, NK], BF16)
            av_sb = const.tile([P, 2, 2, DV + 1], F32)
            dm = const.tile([1, 2], BF16)

            # PSUM discipline: a matmul with start=True wipes its whole
            # bank, so at most one open accumulation group per bank and
            # never reuse a bank while un-copied data sits in it.
            pp = ps_proj.tile([P, 2, NK], F32)        # 1 bank
            sc = ps_sc.tile([P, 3, 512], F32)         # 3 banks, 1 slot each
            avp = ps_av.tile([P, 2, 2, 512], F32)     # 4 banks

            # trigger the exp/tanh ACT table load during the DMA phase
            nc.vector.memset(dm, 0.0)
            nc.scalar.activation(out=dm, in_=dm, func=TANH)

            nc.sync.dma_start(out=wk_sb, in_=wk_d[:, :, :])
            nc.sync.dma_start(out=kin[:, 0], in_=keyT_d[0])
            nc.gpsimd.dma_start(out=wq_sb, in_=wq_d[:, :, :])
            nc.gpsimd.dma_start(out=qin, in_=qryT_d[:, :, :])
            nc.sync.dma_start(out=kin[:, 1], in_=keyT_d[1])
            nc.gpsimd.dma_start(out=wvc_sb, in_=wvc_d[:, :, :])
            nc.gpsimd.dma_start(out=val_sb, in_=val_d[:, :, :])

            # projections: kx_b[h,k] = sum_dk Wk[dk,h]*keyT[dk,k]; same for
            # q. kx0->slice0, then kx1/qx0/qx1 all chain through slice1 so
            # each new group's WAR dep (on the previous copy of slice1)
            # also guarantees every older read of the bank has finished.
            for b in (0, 1):
                for dk in (0, 1):
                    nc.tensor.matmul(pp[:, b, :], wk_sb[:, dk, :],
                                     kin[:, b, dk, :],
                                     start=(dk == 0), stop=(dk == 1))
            nc.vector.tensor_copy(X[:, 0:NK], pp[:, 0, :])
            nc.vector.tensor_copy(X[:, NK:2 * NK], pp[:, 1, :])
            for bi, (qo, qxo, N) in enumerate(((0, 512, N0),
                                               (N0, 512 + N0, N1))):
                for dk in (0, 1):
                    nc.tensor.matmul(pp[:, 1, :N], wq_sb[:, dk, :],
                                     qin[:, dk, qo:qo + N],
                                     start=(dk == 0), stop=(dk == 1))
                nc.vector.tensor_copy(X[:, qxo:qxo + N], pp[:, 1, :N])

            rbounds = []
            r0 = 0
            for gsz in RGROUPS:
                rbounds.append((r0, r0 + gsz))
                r0 += gsz
            assert r0 == R

            def emit_affine(g0, g1):
                for r in range(g0, g1):
                    c, s, t, be, p, w = PARAMS[r]
                    nc.vector.tensor_scalar(
                        out=FT[:, r, 0:512], in0=X[:, 0:512],
                        scalar1=float(s), scalar2=float(t),
                        op0=MULT, op1=ADD)
                    nc.vector.tensor_scalar(
                        out=FT[:, r, 512:L], in0=X[:, 512:L],
                        scalar1=float(p), scalar2=float(w),
                        op0=MULT, op1=ADD)

            emit_affine(*rbounds[0])
            for gi, (g0, g1) in enumerate(rbounds):
                nc.scalar.activation(out=FT[:, g0:g1, :], in_=FT[:, g0:g1, :],
                                     func=TANH)
                if gi + 1 < len(rbounds):
                    emit_affine(*rbounds[gi + 1])
                for r in range(g0, g1):
                    nc.vector.tensor_scalar(
                        out=KFW[:, r, :], in0=FT[:, r, 0:512],
                        scalar1=wvc_sb[:, 0, r:r + 1],
                        scalar2=wvc_sb[:, 1, r:r + 1],
                        op0=MULT, op1=ADD)
                for j, (bi, qb, nn, qo) in enumerate(slots[:3]):
                    for r in range(g0, g1):
                        nc.tensor.matmul(
                            sc[:nn, j, :NK], FT[:, r, qo:qo + nn],
                            KFW[:, r, bi * NK:(bi + 1) * NK],
                            start=(r == 0), stop=(r == R - 1))

            # exp for the first-class slots; slots beyond 3 (only when both
            # batches span 2 q-blocks) accumulate afterwards into the same
            # region as slot j-3, whose exp already consumed it (clean WAR)
            for j in range(min(nqb, 3)):
                nc.scalar.activation(out=attnT[:, j, :], in_=sc[:, j, :NK],
                                     func=EXP)
            for j, (bi, qb, nn, qo) in enumerate(slots):
                if j < 3:
                    continue
                for r in range(R):
                    nc.tensor.matmul(
                        sc[:nn, j - 3, :NK], FT[:, r, qo:qo + nn],
                        KFW[:, r, bi * NK:(bi + 1) * NK],
                        start=(r == 0), stop=(r == R - 1))
                nc.scalar.activation(out=attnT[:, j, :],
                                     in_=sc[:, j - 3, :NK], func=EXP)
            if debug:
                scdbg_sb = const.tile([P, nqb, NK], F32)
                for j in range(min(nqb, 3)):
                    nc.vector.tensor_copy(scdbg_sb[:, j, :], sc[:, j, :NK])
                nc.sync.dma_start(out=xdbg_d[:, :], in_=X)
                nc.sync.dma_start(out=ftdbg_d[:, :, :], in_=FT)
                nc.sync.dma_start(out=kfwdbg_d[:, :, :], in_=KFW)
                nc.sync.dma_start(out=scdbg_d[:, :, :], in_=scdbg_sb)
                nc.sync.dma_start(out=atdbg_d[:, :, :], in_=attnT)

            for bi in (0, 1):
                bslots = [(j, s) for j, s in enumerate(slots) if s[0] == bi]
                for kb in (0, 1):
                    for qi, (j, (_, qb, nn, _)) in enumerate(bslots):
                        nc.tensor.matmul(
                            avp[:, bi, kb, 0:DV + 1],
                            attnT[0:nn, j, kb * P:(kb + 1) * P],
                            val_sb[0:nn, j, :],
                            start=(qi == 0), stop=(qi == len(bslots) - 1))

            for bi in (0, 1):
                for kb in (0, 1):
                    nc.vector.tensor_copy(av_sb[:, bi, kb, :],
                                          avp[:, bi, kb, 0:DV + 1])
                    eng = nc.sync if (bi + kb) % 2 == 0 else nc.gpsimd
                    eng.dma_start(out=av_d[bi, kb], in_=av_sb[:, bi, kb, :])

    nc.compile()
    return nc


def _ceil4(n):
    return -(-int(n) // 4) * 4


def kernel(key, query, value, valid_lens, Wk, Wq, wv, _trace=False):
    key = np.asarray(key, dtype=np.float32)
    query = np.asarray(query, dtype=np.float32)
    value = np.asarray(value, dtype=np.float32)
    Wk = np.asarray(Wk, dtype=np.float32)
    Wq = np.asarray(Wq, dtype=np.float32)
    wv = np.asarray(wv, dtype=np.float32)
    vl = np.clip(np.asarray(valid_lens).astype(np.int64), 1, NQ)

    order = np.argsort(-vl, kind="stable")
    pairs = [(int(order[i]), int(order[B - 1 - i])) for i in range(NCORES)]
    N0 = min(_ceil4(int(vl[order[0]])), NQ)
    N1 = min(_ceil4(int(vl[order[NCORES]])), NQ)

    ckey = (N0, N1)
    if ckey not in _CACHE:
        _CACHE[ckey] = _build(N0, N1)
    nc = _CACHE[ckey]
    slots = _slots(N0, N1)
    nqb = len(slots)

    wk_h = np.ascontiguousarray(
        Wk.reshape(2, P, H).transpose(1, 0, 2)).astype(BF)
    wq_h = np.ascontiguousarray(
        Wq.reshape(2, P, H).transpose(1, 0, 2)).astype(BF)
    prm = np.array(PARAMS, dtype=np.float32)
    wvc = np.empty((P, 2, R), dtype=np.float32)
    wvc[:, 0, :] = wv[:, None] * prm[None, :, 0]    # c_r * wv_h
    wvc[:, 1, :] = wv[:, None] * prm[None, :, 3]    # be_r * wv_h

    def keyT_prep(b):
        return np.ascontiguousarray(
            key[b].T.reshape(2, P, NK).transpose(1, 0, 2)).astype(BF)

    in_maps = []
    for (b0, b1) in pairs:
        qcat = np.zeros((DK, N0 + N1), dtype=np.float32)
        for bi, (b, N, qo) in enumerate(((b0, N0, 0), (b1, N1, N0))):
            n = min(int(vl[b]), N)
            qcat[:, qo:qo + n] = query[b, :n, :].T
        qryT = np.ascontiguousarray(
            qcat.reshape(2, P, N0 + N1).transpose(1, 0, 2)).astype(BF)

        valp = np.zeros((P, nqb, DV + 1), dtype=np.float32)
        for j, (bi, qb, nn, _) in enumerate(slots):
            b = (b0, b1)[bi]
            lo = qb * P
            n = int(np.clip(vl[b] - lo, 0, nn))
            if n > 0:
                valp[:n, j, :DV] = value[b, lo:lo + n, :]
                valp[:n, j, DV] = 1.0

        in_maps.append({
            "keyT": np.stack([keyT_prep(b0), keyT_prep(b1)]),
            "qryT": qryT,
            "val": valp.astype(BF),
            "Wk": wk_h,
            "Wq": wq_h,
            "wvc": wvc,
        })

    res = run_bass_kernel_spmd(nc, in_maps, core_ids=list(range(NCORES)),
                               trace=_trace)
    kernel.last_results = res

    out = np.empty((B, NK, DV), dtype=np.float32)
    for ci, (b0, b1) in enumerate(pairs):
        av = np.asarray(res.results[ci]["av"], dtype=np.float64)
        for bi, b in enumerate((b0, b1)):
            for kb in (0, 1):
                blk = av[bi, kb]
                out[b, kb * P:(kb + 1) * P, :] = (
                    blk[:, :DV] / blk[:, DV:DV + 1]).astype(np.float32)
    return out
